# revision 16
# baseline (speedup 1.0000x reference)
"""Trainium2 Bass kernel for CIN: out[b,m,d] = sigmoid(einsum('bid,bjd,ijm', x0, x, K)).

Shapes (hardcoded): x0,x [4096, 40, 64] f32, kernel [40, 40, 128] f32,
out [4096, 128, 64] f32.

Sharding: data-parallel over batch B across 8 NeuronCores (512 b each).

Per-core pipeline (groups of 8 b's; free dim = 8*64 = 512), all bf16 on
the engines, fp32 accumulation in PSUM:
  - The interaction tensor Z[(i j), (b d)] = x0[i,(b d)] * x[j,(b d)] is
    built directly in matmul-rhs layout ((i j) on partitions) -- no PE
    transposes.  ij-space is blocked 3 i-rows per 128-partition chunk:
    chunk c row p -> (i, j) = (3c + p//40, p%40), p < 120 (8 pad rows).
    14 chunks cover all 40 i's.  Then per chunk:
      * ONE replication matmul with constant 0/1 weights expands x0 rows
        into the chunk's i-pattern: psA[p, bd] = x0T[3c+p//40, bd] (PSUM)
      * the j-side operand B[p, bd] = xT[p%40, bd] is CHUNK-INDEPENDENT,
        host-packed, and DMA'd once per group into SBUF (bf16)
      * one DVE multiply  zc = psA * B  (PSUM x SBUF -> SBUF bf16; only
        one PSUM operand -- DVE has a single PSUM read port)
      * one accumulated matmul  pso += K_c^T @ zc  (contraction (i j))
  - sigmoid fused into PSUM evacuation on ACT, DMA out.

Host-side prep (not on the HW critical path): inputs cast to bf16 and
packed so every DMA is a dense, partition-contiguous load.
"""

import sys

for _p in ("/opt/trn_rl_repo", "/root/.axon_site/_ro/trn_rl_repo"):
    if _p not in sys.path:
        sys.path.insert(0, _p)

from contextlib import ExitStack

import numpy as np
import ml_dtypes

import concourse.bass as bass
from concourse import bacc
import concourse.tile as tile
from concourse import mybir
from concourse.bass_utils import run_bass_kernel_spmd

B, F0, F, D, M = 4096, 40, 40, 64, 128
NCORES = 8
NB = B // NCORES            # 512 b per core
GB = 8                      # b's per group
FREE = GB * D               # 512 = matmul free dim = one PSUM bank (f32)
NG = NB // GB               # 64 groups per core
IJ = F0 * F                 # 1600
IPC = 3                     # i-rows per chunk
ROWS = IPC * F              # 120 valid rows per chunk
NCHUNK = (F0 + IPC - 1) // IPC  # 14

f32 = mybir.dt.float32
bf16 = mybir.dt.bfloat16
BF16 = ml_dtypes.bfloat16


def _pack_kernel(kernel_np: np.ndarray) -> np.ndarray:
    """K[i,j,m] -> kwT [128, NCHUNK, M] bf16,
    kwT[p, c, m] = K[3c + p//40, p%40, m] (zero where invalid)."""
    kf = np.zeros((NCHUNK, 128, M), dtype=np.float32)
    p = np.arange(ROWS)
    for c in range(NCHUNK):
        i = IPC * c + p // F
        valid = i < F0
        kf[c, p[valid]] = kernel_np[i[valid], p[valid] % F]
    return np.ascontiguousarray(kf.transpose(1, 0, 2).astype(BF16))


def _pack_reps() -> np.ndarray:
    """Constant replication weights [F0, NCHUNK, 128] bf16:
    rp[k, c, p] = (k == 3c + p//40), p < 120."""
    rp = np.zeros((F0, NCHUNK, 128), dtype=np.float32)
    p = np.arange(ROWS)
    for c in range(NCHUNK):
        i = IPC * c + p // F
        valid = i < F0
        rp[i[valid], c, p[valid]] = 1.0
    return np.ascontiguousarray(rp.astype(BF16))


def _pack_x(x0: np.ndarray) -> np.ndarray:
    """-> xp [NCORES, NG, F0, FREE] bf16: x0T per (core, group)."""
    x0r = x0.reshape(NCORES, NG, GB, F0, D).transpose(0, 1, 3, 2, 4)
    return np.ascontiguousarray(
        x0r.reshape(NCORES, NG, F0, FREE).astype(BF16))


def _pack_b(x: np.ndarray) -> np.ndarray:
    """-> bp [NCORES, NG, 128, FREE] bf16: B[p, bd] = xT[p%40, bd] for
    p < 120, zero pad rows."""
    xr = x.reshape(NCORES, NG, GB, F, D).transpose(0, 1, 3, 2, 4)
    xr = xr.reshape(NCORES, NG, F, FREE).astype(BF16)
    bp = np.zeros((NCORES, NG, 128, FREE), dtype=BF16)
    bp[:, :, 0:ROWS, :] = np.concatenate([xr] * IPC, axis=2)
    return bp


def _build(nb: int):
    ng = nb // GB

    nc = bacc.Bacc("TRN2", num_devices=8)
    xp = nc.declare_dram_parameter("xp", [ng, F0, FREE], bf16, isOutput=False)
    bpp = nc.declare_dram_parameter("bp", [ng, 128, FREE], bf16, isOutput=False)
    kp = nc.declare_dram_parameter("kp", [128, NCHUNK, M], bf16, isOutput=False)
    rep = nc.declare_dram_parameter("rep", [F0, NCHUNK, 128], bf16, isOutput=False)
    outp = nc.declare_dram_parameter("out", [nb, M, D], f32, isOutput=True)

    with ExitStack() as ctx:
        tc = ctx.enter_context(tile.TileContext(nc))
        singles = ctx.enter_context(tc.tile_pool(name="singles", bufs=1))
        xx_pool = ctx.enter_context(tc.tile_pool(name="xx", bufs=3))
        bb_pool = ctx.enter_context(tc.tile_pool(name="bb", bufs=3))
        zc_pool = ctx.enter_context(tc.tile_pool(name="zc", bufs=6))
        ea_pool = ctx.enter_context(tc.tile_pool(name="ea", bufs=4))
        eh_pool = ctx.enter_context(tc.tile_pool(name="eh", bufs=2))
        osb_pool = ctx.enter_context(tc.tile_pool(name="osb", bufs=3))
        psa_pool = ctx.enter_context(tc.tile_pool(name="psa", bufs=3, space="PSUM"))
        pso_pool = ctx.enter_context(tc.tile_pool(name="pso", bufs=2, space="PSUM"))

        kw = singles.tile([128, NCHUNK, M], bf16)
        nc.sync.dma_start(out=kw, in_=kp[:])
        rp = singles.tile([F0, NCHUNK, 128], bf16)
        nc.sync.dma_start(out=rp, in_=rep[:])

        # HAM warm-up spin: ~16 dense back-to-back matmuls (~5us) raise the
        # PE clock-gate to K=8/8 (2.4 GHz) before the real work starts.  The
        # steady-state loop never leaves the PE idle >3.4us, so it stays
        # warm for the whole kernel.  Without this the PE runs at 1.2 GHz
        # throughout (per-chunk DVE waits prevent a 3.4us busy window).
        spin_w = singles.tile([128, 128], bf16)
        nc.vector.memset(spin_w, 0.0)
        spin_r = singles.tile([128, FREE], bf16)
        nc.vector.memset(spin_r, 0.0)
        ps_spin = pso_pool.tile([128, FREE], f32, tag="pso")
        for _ in range(10):
            nc.tensor.matmul(ps_spin, spin_w, spin_r, start=True, stop=True)

        for g in range(ng):
            bsl = slice(g * GB, (g + 1) * GB)
            xx = xx_pool.tile([F0, FREE], bf16, tag="xx")
            nc.sync.dma_start(out=xx, in_=xp[g])
            bb = bb_pool.tile([128, FREE], bf16, tag="bb")
            nc.sync.dma_start(out=bb, in_=bpp[g])

            pso = pso_pool.tile([128, FREE], f32, tag="pso")
            # chunks in pairs: both rep outputs land in one 2-bank PSUM
            # tile so a single DVE multiply (FD=1024) covers 2 chunks --
            # amortizes the DVE per-instruction + PSUM-port overhead.
            for q in range(NCHUNK // 2):
                psA = psa_pool.tile([128, 2, FREE], f32, tag="psa")
                nc.tensor.matmul(psA[:, 0, :], rp[:, 2 * q, :], xx,
                                 start=True, stop=True)
                nc.tensor.matmul(psA[:, 1, :], rp[:, 2 * q + 1, :], xx,
                                 start=True, stop=True)
                zc = zc_pool.tile([128, 2, FREE], bf16, tag="zc")
                if q in (1, 3, 5):
                    # ACT pre-evacuates psA to SBUF bf16 through its own
                    # PSUM port; the DVE multiplies then run in 2x mode
                    # (both operands SBUF bf16, no broadcast).
                    ea = ea_pool.tile([128, 2, FREE], bf16, tag="ea")
                    nc.scalar.copy(out=ea, in_=psA)
                    nc.vector.tensor_tensor(out=zc[:, 0, :], in0=ea[:, 0, :],
                                            in1=bb, op=mybir.AluOpType.mult)
                    nc.vector.tensor_tensor(out=zc[:, 1, :], in0=ea[:, 1, :],
                                            in1=bb, op=mybir.AluOpType.mult)
                elif q == 6:
                    # split pair: ACT evacuates one half (its PSUM port),
                    # giving the DVE one 2x multiply + one 1x PSUM multiply
                    eh = eh_pool.tile([128, FREE], bf16, tag="eh")
                    nc.scalar.copy(out=eh, in_=psA[:, 1, :])
                    nc.vector.tensor_tensor(out=zc[:, 0, :], in0=psA[:, 0, :],
                                            in1=bb, op=mybir.AluOpType.mult)
                    nc.vector.tensor_tensor(out=zc[:, 1, :], in0=eh,
                                            in1=bb, op=mybir.AluOpType.mult)
                else:
                    nc.vector.tensor_tensor(
                        out=zc, in0=psA,
                        in1=bb.unsqueeze(1).broadcast_to((128, 2, FREE)),
                        op=mybir.AluOpType.mult)
                nc.tensor.matmul(pso, kw[:, 2 * q, :], zc[:, 0, :],
                                 start=(q == 0), stop=False)
                nc.tensor.matmul(pso, kw[:, 2 * q + 1, :], zc[:, 1, :],
                                 start=False, stop=(q == NCHUNK // 2 - 1))

            osb = osb_pool.tile([128, GB, D], f32, tag="osb")
            nc.scalar.activation(osb.rearrange("m b d -> m (b d)"), pso,
                                 mybir.ActivationFunctionType.Sigmoid)
            nc.sync.dma_start(out=outp[bsl].transpose([1, 0, 2]), in_=osb)

    nc.finalize()
    return nc


_NC_CACHE = {}


def _get_nc():
    if "nc" not in _NC_CACHE:
        _NC_CACHE["nc"] = _build(NB)
    return _NC_CACHE["nc"]


def _make_in_maps(x0: np.ndarray, x: np.ndarray, kernel: np.ndarray):
    x0 = np.ascontiguousarray(np.asarray(x0, dtype=np.float32))
    x = np.ascontiguousarray(np.asarray(x, dtype=np.float32))
    kw = _pack_kernel(np.asarray(kernel, dtype=np.float32))
    rp = _pack_reps()
    xp = _pack_x(x0)
    bp = _pack_b(x)
    return [
        {"xp": xp[i], "bp": bp[i], "kp": kw, "rep": rp}
        for i in range(NCORES)
    ]


def kernel(x0: np.ndarray, x: np.ndarray, kernel: np.ndarray) -> np.ndarray:
    nc = _get_nc()
    in_maps = _make_in_maps(x0, x, kernel)
    res = run_bass_kernel_spmd(nc, in_maps, list(range(NCORES)))
    out = np.concatenate([np.asarray(r["out"]) for r in res.results], axis=0)
    return out.astype(np.float32)


# revision 17
# speedup vs baseline: 1.7426x; 1.7426x over previous
"""Trainium2 Bass kernel for CIN: out[b,m,d] = sigmoid(einsum('bid,bjd,ijm', x0, x, K)).

Shapes (hardcoded): x0,x [4096, 40, 64] f32, kernel [40, 40, 128] f32,
out [4096, 128, 64] f32.

Sharding: data-parallel over batch B across 8 NeuronCores (512 b each).

Per-core pipeline (groups of 8 b's; free dim = 8*64 = 512), all bf16 on
the engines, fp32 accumulation in PSUM:
  - The interaction tensor Z[(i j), (b d)] = x0[i,(b d)] * x[j,(b d)] is
    built directly in matmul-rhs layout ((i j) on partitions) -- no PE
    transposes.  ij-space is blocked 3 i-rows per 128-partition chunk:
    chunk c row p -> (i, j) = (3c + p//40, p%40), p < 120 (8 pad rows).
    14 chunks cover all 40 i's.  Then per chunk:
      * ONE replication matmul with constant 0/1 weights expands x0 rows
        into the chunk's i-pattern: psA[p, bd] = x0T[3c+p//40, bd] (PSUM)
      * the j-side operand B[p, bd] = xT[p%40, bd] is CHUNK-INDEPENDENT,
        host-packed, and DMA'd once per group into SBUF (bf16)
      * one DVE multiply  zc = psA * B  (PSUM x SBUF -> SBUF bf16; only
        one PSUM operand -- DVE has a single PSUM read port)
      * one accumulated matmul  pso += K_c^T @ zc  (contraction (i j))
  - sigmoid fused into PSUM evacuation on ACT, DMA out.

Host-side prep (not on the HW critical path): inputs cast to bf16 and
packed so every DMA is a dense, partition-contiguous load.
"""

import sys

for _p in ("/opt/trn_rl_repo", "/root/.axon_site/_ro/trn_rl_repo"):
    if _p not in sys.path:
        sys.path.insert(0, _p)

from contextlib import ExitStack

import numpy as np
import ml_dtypes

import concourse.bass as bass
from concourse import bacc
import concourse.tile as tile
from concourse import mybir
from concourse.bass_utils import run_bass_kernel_spmd

B, F0, F, D, M = 4096, 40, 40, 64, 128
NCORES = 8
NB = B // NCORES            # 512 b per core
GB = 8                      # b's per group
FREE = GB * D               # 512 = matmul free dim = one PSUM bank (f32)
NG = NB // GB               # 64 groups per core
IJ = F0 * F                 # 1600
IPC = 3                     # i-rows per chunk
ROWS = IPC * F              # 120 valid rows per chunk
NCHUNK = (F0 + IPC - 1) // IPC  # 14

f32 = mybir.dt.float32
bf16 = mybir.dt.bfloat16
BF16 = ml_dtypes.bfloat16


def _pack_kernel(kernel_np: np.ndarray) -> np.ndarray:
    """K[i,j,m] -> kwT [128, NCHUNK, M] bf16,
    kwT[p, c, m] = K[3c + p//40, p%40, m] (zero where invalid)."""
    kf = np.zeros((NCHUNK, 128, M), dtype=np.float32)
    p = np.arange(ROWS)
    for c in range(NCHUNK):
        i = IPC * c + p // F
        valid = i < F0
        kf[c, p[valid]] = kernel_np[i[valid], p[valid] % F]
    return np.ascontiguousarray(kf.transpose(1, 0, 2).astype(BF16))


def _pack_reps() -> np.ndarray:
    """Constant replication weights [F0, NCHUNK, 128] bf16:
    rp[k, c, p] = (k == 3c + p//40), p < 120."""
    rp = np.zeros((F0, NCHUNK, 128), dtype=np.float32)
    p = np.arange(ROWS)
    for c in range(NCHUNK):
        i = IPC * c + p // F
        valid = i < F0
        rp[i[valid], c, p[valid]] = 1.0
    return np.ascontiguousarray(rp.astype(BF16))


def _pack_x(x0: np.ndarray) -> np.ndarray:
    """-> xp [NCORES, NG, F0, FREE] bf16: x0T per (core, group)."""
    x0r = x0.reshape(NCORES, NG, GB, F0, D).transpose(0, 1, 3, 2, 4)
    return np.ascontiguousarray(
        x0r.reshape(NCORES, NG, F0, FREE).astype(BF16))


def _pack_b(x: np.ndarray) -> np.ndarray:
    """-> bp [NCORES, NG, 128, FREE] bf16: B[p, bd] = xT[p%40, bd] for
    p < 120, zero pad rows."""
    xr = x.reshape(NCORES, NG, GB, F, D).transpose(0, 1, 3, 2, 4)
    xr = xr.reshape(NCORES, NG, F, FREE).astype(BF16)
    bp = np.zeros((NCORES, NG, 128, FREE), dtype=BF16)
    bp[:, :, 0:ROWS, :] = np.concatenate([xr] * IPC, axis=2)
    return bp


def _build(nb: int):
    ng = nb // GB

    nc = bacc.Bacc("TRN2", num_devices=8)
    xp = nc.declare_dram_parameter("xp", [ng, F0, FREE], bf16, isOutput=False)
    bpp = nc.declare_dram_parameter("bp", [ng, 128, FREE], bf16, isOutput=False)
    kp = nc.declare_dram_parameter("kp", [128, NCHUNK, M], bf16, isOutput=False)
    rep = nc.declare_dram_parameter("rep", [F0, NCHUNK, 128], bf16, isOutput=False)
    outp = nc.declare_dram_parameter("out", [nb, M, D], f32, isOutput=True)

    with ExitStack() as ctx:
        tc = ctx.enter_context(tile.TileContext(nc))
        singles = ctx.enter_context(tc.tile_pool(name="singles", bufs=1))
        xx_pool = ctx.enter_context(tc.tile_pool(name="xx", bufs=3))
        bb_pool = ctx.enter_context(tc.tile_pool(name="bb", bufs=3))
        zc_pool = ctx.enter_context(tc.tile_pool(name="zc", bufs=4))
        ea_pool = ctx.enter_context(tc.tile_pool(name="ea", bufs=3))
        osb_pool = ctx.enter_context(tc.tile_pool(name="osb", bufs=3))
        psa_pool = ctx.enter_context(tc.tile_pool(name="psa", bufs=3, space="PSUM"))
        pso_pool = ctx.enter_context(tc.tile_pool(name="pso", bufs=2, space="PSUM"))

        kw = singles.tile([128, NCHUNK, M], bf16)
        nc.sync.dma_start(out=kw, in_=kp[:])
        rp = singles.tile([F0, NCHUNK, 128], bf16)
        nc.sync.dma_start(out=rp, in_=rep[:])

        # HAM warm-up spin: ~16 dense back-to-back matmuls (~5us) raise the
        # PE clock-gate to K=8/8 (2.4 GHz) before the real work starts.  The
        # steady-state loop never leaves the PE idle >3.4us, so it stays
        # warm for the whole kernel.  Without this the PE runs at 1.2 GHz
        # throughout (per-chunk DVE waits prevent a 3.4us busy window).
        spin_w = singles.tile([128, 128], bf16)
        nc.vector.memset(spin_w, 0.0)
        spin_r = singles.tile([128, FREE], bf16)
        nc.vector.memset(spin_r, 0.0)
        ps_spin = pso_pool.tile([128, FREE], f32, tag="pso")
        for _ in range(10):
            nc.tensor.matmul(ps_spin, spin_w, spin_r, start=True, stop=True)

        for g in range(ng):
            bsl = slice(g * GB, (g + 1) * GB)
            xx = xx_pool.tile([F0, FREE], bf16, tag="xx")
            nc.sync.dma_start(out=xx, in_=xp[g])
            bb = bb_pool.tile([128, FREE], bf16, tag="bb")
            nc.sync.dma_start(out=bb, in_=bpp[g])

            pso = pso_pool.tile([128, FREE], f32, tag="pso")
            # chunks in pairs: both rep outputs land in one 2-bank PSUM
            # tile so a single DVE multiply (FD=1024) covers 2 chunks --
            # amortizes the DVE per-instruction + PSUM-port overhead.
            for q in range(NCHUNK // 2):
                psA = psa_pool.tile([128, 2, FREE], f32, tag="psa")
                nc.tensor.matmul(psA[:, 0, :], rp[:, 2 * q, :], xx,
                                 start=True, stop=True)
                nc.tensor.matmul(psA[:, 1, :], rp[:, 2 * q + 1, :], xx,
                                 start=True, stop=True)
                zc = zc_pool.tile([128, 2, FREE], bf16, tag="zc")
                if q in (1, 3, 5):
                    # ACT pre-evacuates psA to SBUF bf16 through its own
                    # PSUM port; the DVE multiplies then run in 2x mode
                    # (both operands SBUF bf16, no broadcast).
                    ea = ea_pool.tile([128, 2, FREE], bf16, tag="ea")
                    nc.scalar.copy(out=ea, in_=psA)
                    nc.vector.tensor_tensor(out=zc[:, 0, :], in0=ea[:, 0, :],
                                            in1=bb, op=mybir.AluOpType.mult)
                    nc.vector.tensor_tensor(out=zc[:, 1, :], in0=ea[:, 1, :],
                                            in1=bb, op=mybir.AluOpType.mult)
                else:
                    nc.vector.tensor_tensor(
                        out=zc, in0=psA,
                        in1=bb.unsqueeze(1).broadcast_to((128, 2, FREE)),
                        op=mybir.AluOpType.mult)
                nc.tensor.matmul(pso, kw[:, 2 * q, :], zc[:, 0, :],
                                 start=(q == 0), stop=False)
                nc.tensor.matmul(pso, kw[:, 2 * q + 1, :], zc[:, 1, :],
                                 start=False, stop=(q == NCHUNK // 2 - 1))

            osb = osb_pool.tile([128, GB, D], f32, tag="osb")
            nc.scalar.activation(osb.rearrange("m b d -> m (b d)"), pso,
                                 mybir.ActivationFunctionType.Sigmoid)
            nc.sync.dma_start(out=outp[bsl].transpose([1, 0, 2]), in_=osb)

    nc.finalize()
    return nc


_NC_CACHE = {}


def _get_nc():
    if "nc" not in _NC_CACHE:
        _NC_CACHE["nc"] = _build(NB)
    return _NC_CACHE["nc"]


def _make_in_maps(x0: np.ndarray, x: np.ndarray, kernel: np.ndarray):
    x0 = np.ascontiguousarray(np.asarray(x0, dtype=np.float32))
    x = np.ascontiguousarray(np.asarray(x, dtype=np.float32))
    kw = _pack_kernel(np.asarray(kernel, dtype=np.float32))
    rp = _pack_reps()
    xp = _pack_x(x0)
    bp = _pack_b(x)
    return [
        {"xp": xp[i], "bp": bp[i], "kp": kw, "rep": rp}
        for i in range(NCORES)
    ]


def kernel(x0: np.ndarray, x: np.ndarray, kernel: np.ndarray) -> np.ndarray:
    nc = _get_nc()
    in_maps = _make_in_maps(x0, x, kernel)
    res = run_bass_kernel_spmd(nc, in_maps, list(range(NCORES)))
    out = np.concatenate([np.asarray(r["out"]) for r in res.results], axis=0)
    return out.astype(np.float32)
